# revision 1
# baseline (speedup 1.0000x reference)
"""Trainium2 Bass kernel for nn_Decoder (moe_routing, 4-species expert decoder).

Reference semantics (per species i, m = 4096 entries; only the first 512
decoded rows are ever read because decoded[bi, gi] indexes rows with *cell*
ids < 512):

    bi   = batch_idx[i*m:(i+1)*m]            # cell ids < 512
    gi   = gene_idx[i*m:(i+1)*m]
    comb = concat(z[i][:512], global_latent[bi[:512]])       # [512, 512]
    h1   = relu(comb @ W1[i] + b1[i])                        # [512, 1024]
    h2   = relu(h1 @ W2[i] + b2[i])                          # [512, 1024]
    out[e] = softplus(h2[bi[e]] . W3[i][:, gi[e]] + b3[i][gi[e]])

Sharding: expert-parallel x entry-parallel.  Core c handles species c//2 and
entries [(c%2)*2048, (c%2+1)*2048) of that species.  Each core computes the
512-row MLP for its species, then per-entry dot products via two row-gathers
(h2 rows by cell id, W3^T rows by gene id) and a mul+reduce, with reductions
split across the Vector and Scalar engines.  Entries are routed by cell
quartile on the host so each h2-gather only depends on one quarter of h2.
Math in bf16 with f32 accumulation.
"""

import os
import sys

import numpy as np

for _p in ("/root/.axon_site/_ro/trn_rl_repo", "/opt/trn_rl_repo"):
    if os.path.isdir(_p) and _p not in sys.path:
        sys.path.append(_p)

import ml_dtypes

BF = ml_dtypes.bfloat16

N_SPECIES = 4
NNZ = 16384
N_CELLS = 512
L = 256          # latent
H = 1024         # hidden
G = 20000        # genes
M = NNZ // N_SPECIES   # 4096 entries per species
R = 512          # decoded rows actually used per species
NE = 2048        # entries per core
GP = 640         # per-cell-quartile group, padded (worst observed 561)
NT = 4 * GP // 128     # = 20 dot chunks of 128 entries
N_CORES = 8

_NC = None            # cached compiled Bass module
LAST_RESULTS = None   # BassKernelResults of the last run (for profiling)


def _build_nc():
    from contextlib import ExitStack

    import concourse.bacc as bacc
    import concourse.mybir as mybir
    import concourse.tile as tile

    F32 = mybir.dt.float32
    BF16 = mybir.dt.bfloat16
    I16 = mybir.dt.int16
    AF = mybir.ActivationFunctionType
    OP = mybir.AluOpType

    nc = bacc.Bacc(None, target_bir_lowering=False, num_swdge_queues=4,
                   dynamic_dma_scratch_size=32768)

    w1 = nc.dram_tensor("w1", [128, 4, H], BF16, kind="ExternalInput")
    w2 = nc.dram_tensor("w2", [128, 8, H], BF16, kind="ExternalInput")
    zt = nc.dram_tensor("zt", [128, 2, R], BF16, kind="ExternalInput")
    glk = nc.dram_tensor("glk", [128, 4, L], BF16, kind="ExternalInput")
    pt_in = nc.dram_tensor("pt_in", [128, 4, R], BF16, kind="ExternalInput")
    w3t = nc.dram_tensor("w3t", [G, H], BF16, kind="ExternalInput")
    b1s = nc.dram_tensor("b1s", [128, 8], F32, kind="ExternalInput")
    b2r = nc.dram_tensor("b2r", [1, H], BF16, kind="ExternalInput")
    ones1 = nc.dram_tensor("ones1", [1, 128], BF16, kind="ExternalInput")
    b3g = nc.dram_tensor("b3g", [128, NT], F32, kind="ExternalInput")
    biEw = nc.dram_tensor("biEw", [128, 4 * GP // 16], I16, kind="ExternalInput")
    giEw = nc.dram_tensor("giEw", [128, 4 * GP // 16], I16, kind="ExternalInput")
    out = nc.dram_tensor("out", [128, NT], F32, kind="ExternalOutput")

    gcols = GP // 16   # idx columns per group (wrapped 16-way)
    gts = GP // 128    # dot chunks per group

    with tile.TileContext(nc) as tc, ExitStack() as ctx:
        const = ctx.enter_context(tc.tile_pool(name="const", bufs=1))
        work = ctx.enter_context(tc.tile_pool(name="work", bufs=1))
        prodp = ctx.enter_context(tc.tile_pool(name="prod", bufs=3))
        psum = ctx.enter_context(tc.tile_pool(name="psum", bufs=2, space="PSUM"))
        dram = ctx.enter_context(tc.tile_pool(name="dram", bufs=1, space="DRAM"))

        def load(src, shape, dtype, tag, eng=None):
            t = const.tile(shape, dtype, tag=tag)
            (eng or nc.sync).dma_start(t[:], src[:])
            return t

        # Index array first (unblocks the W3 gathers), then the small
        # combT-path inputs on the sync queue; the weight tables go on the
        # scalar HWDGE queue in parallel, split into <=512KB pieces so no
        # small load's completion gets semaphore-lane-chained behind a
        # multi-microsecond transfer.
        giEw_s = load(giEw, [128, 4 * GP // 16], I16, "giEw")
        w1_s = const.tile([128, 4, H], BF16, tag="w1")
        for k2 in range(2):
            nc.scalar.dma_start(w1_s[:, 2 * k2 : 2 * (k2 + 1), :],
                                w1[:, 2 * k2 : 2 * (k2 + 1), :])
        w2_s = const.tile([128, 8, H], BF16, tag="w2")
        for k2 in range(4):
            nc.scalar.dma_start(w2_s[:, 2 * k2 : 2 * (k2 + 1), :],
                                w2[:, 2 * k2 : 2 * (k2 + 1), :])
        pt_s = load(pt_in, [128, 4, R], BF16, "pt_s")
        glk_s = load(glk, [128, 4, L], BF16, "glk")
        biEw_s = load(biEw, [128, 4 * GP // 16], I16, "biEw")
        b1_s = load(b1s, [128, 8], F32, "b1")
        b2_s = load(b2r, [1, H], BF16, "b2")
        one_s = load(ones1, [1, 128], BF16, "ones")
        b3g_s = load(b3g, [128, NT], F32, "b3g")

        # W3^T row gather by gene id (entry-permuted order), one tile per
        # cell-quartile group: wgs[g][p, u, :] = W3T[giE[(gts*g+u)*128+p], :]
        wgs = []
        for g in range(4):
            wgt = work.tile([128, GP // 128, H], BF16, name=f"wg{g}",
                            tag=f"wg{g}")
            nc.gpsimd.dma_gather(
                out_ap=wgt[:],
                in_ap=w3t[:],
                idxs_ap=giEw_s[:, (GP // 16) * g : (GP // 16) * (g + 1)],
                num_idxs=GP,
                num_idxs_reg=GP,
                elem_size=H,
                queue_num=g % 2,
            )
            wgs.append(wgt)

        # combT: [512 features, 512 rows] as 4 k-tiles.  k0,k1 = z^T (direct
        # load); k2,k3 = global_latent^T[bi] built on the PE as gl.T @ onehot
        # (host-uploaded one-hot P[c, r] = (bi[r] == c)).
        combT = const.tile([128, 4, R], BF16, tag="combT")
        nc.sync.dma_start(combT[:, 0:2, :], zt[:])
        for c2 in range(2):
            pc = psum.tile([128, R], F32, tag="pc")
            for kt in range(4):
                nc.tensor.matmul(
                    pc[:],
                    glk_s[:, kt, c2 * 128 : (c2 + 1) * 128],
                    pt_s[:, kt, :],
                    start=(kt == 0),
                    stop=(kt == 3),
                )
            nc.scalar.activation(combT[:, 2 + c2, :], pc[:], AF.Copy)

        # h1T[h, rows]: out = W1_slice.T @ combT, relu + per-partition b1.
        h1T = work.tile([128, 8, R], BF16, tag="h1T")
        for mt in range(8):
            ps = psum.tile([128, R], F32, tag="ps1")
            for kt in range(4):
                nc.tensor.matmul(
                    ps[:],
                    w1_s[:, kt, mt * 128 : (mt + 1) * 128],
                    combT[:, kt, :],
                    start=(kt == 0),
                    stop=(kt == 3),
                )
            nc.scalar.activation(
                h1T[:, mt, :], ps[:], AF.Relu, bias=b1_s[:, mt : mt + 1]
            )

        # h2 in row layout per cell-quartile: out = h1T_slice.T @ W2 (+ bias
        # via ones.T @ b2).  Each quartile streams to its own DRAM table so
        # its h2-gather can start before the other quartiles finish.
        h2row = work.tile([128, 4, H], BF16, tag="h2row")
        h2ds = [
            dram.tile([128, H], BF16, name=f"h2d{g}", tag=f"h2d{g}")
            for g in range(4)
        ]
        dots = work.tile([128, NT], F32, tag="dots")
        hgs = []
        relu_insts = []

        for mt in range(4):
            for nt in range(2):
                ps = psum.tile([128, 512], F32, tag="ps2")
                for kt in range(8):
                    nc.tensor.matmul(
                        ps[:],
                        h1T[:, kt, mt * 128 : (mt + 1) * 128],
                        w2_s[:, kt, nt * 512 : (nt + 1) * 512],
                        start=(kt == 0),
                        stop=False,
                    )
                nc.tensor.matmul(
                    ps[:],
                    one_s[:],
                    b2_s[:, nt * 512 : (nt + 1) * 512],
                    start=False,
                    stop=True,
                )
                relu_insts.append(nc.scalar.activation(
                    h2row[:, mt, nt * 512 : (nt + 1) * 512], ps[:], AF.Relu
                ))
            nc.sync.dma_start(h2ds[mt][:], h2row[:, mt, :])

            # This quartile's entries: gather h2 rows (local cell ids).
            hgt = work.tile([128, GP // 128, H], BF16, name=f"hg{mt}",
                            tag=f"hg{mt}")
            nc.gpsimd.dma_gather(
                out_ap=hgt[:],
                in_ap=h2ds[mt][:],
                idxs_ap=biEw_s[:, gcols * mt : gcols * (mt + 1)],
                num_idxs=GP,
                num_idxs_reg=GP,
                elem_size=H,
                queue_num=(1, 2, 1, 3)[mt],
            )
            hgs.append(hgt)

        # Per-entry dots, after ALL MLP/relu work is emitted so the reduce
        # COPYs (which wait on gather data) can't delay the relus in the
        # Scalar engine's instruction order.
        from concourse.tile_rust import add_dep_helper

        for mt in range(4):
            for u in range(gts):
                t = gts * mt + u
                pr = prodp.tile([128, H], BF16, tag="pr")
                nc.vector.tensor_tensor(pr[:], hgs[mt][:, u, :],
                                        wgs[mt][:, u, :], OP.mult)
                if t % 2 == 0:
                    nc.vector.tensor_reduce(
                        dots[:, t : t + 1], pr[:], mybir.AxisListType.X, OP.add
                    )
                else:
                    prc = prodp.tile([128, H], BF16, tag="prc")
                    cp = nc.scalar.activation(
                        prc[:], pr[:], AF.Copy, accum_out=dots[:, t : t + 1]
                    )
                    # The scheduler's cost model underestimates gather time;
                    # without this it slots gather-gated copies ahead of the
                    # final relus in the Scalar FIFO, stalling the last h2
                    # write by ~20us.
                    add_dep_helper(cp.ins, relu_insts[-1].ins, sync=False,
                                   reason="dots copies after all relus")
        nc.vector.tensor_tensor(dots[:], dots[:], b3g_s[:], OP.add)

        # softplus(x) = ln(1 + e^x).  No ln/softplus in the HW act tables, so
        # compute u = e^x, y = u + 1, then ln(y) by Newton on f(l) = e^l - y:
        # l <- l + y*e^(-l) - 1, seeded with the Pade estimate 2u/(u+2).
        u = work.tile([128, NT], F32, tag="u")
        y = work.tile([128, NT], F32, tag="y")
        r = work.tile([128, NT], F32, tag="r")
        l = work.tile([128, NT], F32, tag="l")
        t_ = work.tile([128, NT], F32, tag="t_")
        nc.scalar.activation(u[:], dots[:], AF.Exp)
        nc.vector.tensor_scalar_add(y[:], u[:], 1.0)
        nc.vector.tensor_scalar_add(r[:], u[:], 2.0)
        with nc.allow_low_precision("newton seed only"):
            nc.vector.reciprocal(r[:], r[:])
        nc.vector.tensor_tensor(l[:], u[:], r[:], OP.mult)
        nc.vector.tensor_scalar_mul(l[:], l[:], 2.0)
        for _ in range(2):
            nc.scalar.activation(t_[:], l[:], AF.Exp, scale=-1.0)
            nc.vector.tensor_tensor(t_[:], y[:], t_[:], OP.mult)
            nc.vector.tensor_scalar_add(t_[:], t_[:], -1.0)
            nc.vector.tensor_tensor(l[:], l[:], t_[:], OP.add)
        nc.sync.dma_start(out[:], l[:])

    nc.finalize()
    return nc


def _get_nc():
    global _NC
    if _NC is None:
        _NC = _build_nc()
    return _NC


def _wrap_idx(a):
    """Wrap an index vector into the SWDGE layout: idx j at [j%16, j//16],
    replicated across the 8 gpsimd cores' partition groups -> [128, n//16]."""
    a = np.asarray(a, dtype=np.int16)
    w = a.reshape(-1, 16).T  # [16, n//16]
    return np.ascontiguousarray(np.tile(w, (8, 1)))


def _prep_core_inputs(c, batch_idx, gene_idx, global_latent, z, W1, b1, W2, b2,
                      b3, w3t_bf):
    """Build the device input map for core c plus the slot->global-entry map
    used to assemble the output (slot s = t*128 + p; -1 = padding)."""
    i, j = c // 2, c % 2
    base = i * M + j * NE
    biE_np = np.asarray(batch_idx[base : base + NE], dtype=np.int64)
    giE_np = np.asarray(gene_idx[base : base + NE], dtype=np.int64)
    bi512_np = np.asarray(batch_idx[i * M : i * M + R], dtype=np.int64)

    # Route entries by cell quartile; pad each group to GP entries.
    slot_entry = np.full(4 * GP, -1, dtype=np.int64)
    bi_loc = np.zeros(4 * GP, dtype=np.int16)
    gi_perm = np.zeros(4 * GP, dtype=np.int16)
    b3_perm = np.zeros(4 * GP, dtype=np.float32)
    gts = GP // 128
    for g in range(4):
        eg = np.nonzero(biE_np // 128 == g)[0]
        assert len(eg) <= GP, f"cell-quartile group overflow: {len(eg)} > {GP}"
        ii = np.arange(len(eg))
        slots = (gts * g + ii // 128) * 128 + ii % 128
        slot_entry[slots] = base + eg
        gslice = slice(g * GP, (g + 1) * GP)
        bi_loc[gslice][: len(eg)] = (biE_np[eg] - 128 * g).astype(np.int16)
        gi_perm[gslice][: len(eg)] = giE_np[eg].astype(np.int16)
        b3_perm[gslice][: len(eg)] = b3[i][giE_np[eg]]

    # b3 in slot layout [128, NT]
    b3g = np.zeros((128, NT), dtype=np.float32)
    for g in range(4):
        blk = b3_perm[g * GP : (g + 1) * GP].reshape(gts, 128).T
        b3g[:, gts * g : gts * (g + 1)] = blk

    biEw = np.concatenate(
        [_wrap_idx(bi_loc[g * GP : (g + 1) * GP]) for g in range(4)], axis=1)
    giEw = np.concatenate(
        [_wrap_idx(gi_perm[g * GP : (g + 1) * GP]) for g in range(4)], axis=1)

    zt = np.ascontiguousarray(z[i, :R].T)  # [256, 512]
    # one-hot routing matrix P[c, r] = (bi512[r] == c), tiled [128, 4, 512]
    pt = np.zeros((N_CELLS, R), dtype=BF)
    pt[bi512_np, np.arange(R)] = 1
    in_map = {
        "w1": np.ascontiguousarray(
            W1[i].reshape(4, 128, H).transpose(1, 0, 2)).astype(BF),
        "w2": np.ascontiguousarray(
            W2[i].reshape(8, 128, H).transpose(1, 0, 2)).astype(BF),
        "zt": np.ascontiguousarray(
            zt.reshape(2, 128, R).transpose(1, 0, 2)).astype(BF),
        "glk": np.ascontiguousarray(
            global_latent.reshape(4, 128, L).transpose(1, 0, 2)).astype(BF),
        "pt_in": np.ascontiguousarray(
            pt.reshape(4, 128, R).transpose(1, 0, 2)),
        "w3t": w3t_bf[i],
        "b1s": np.ascontiguousarray(b1[i].reshape(8, 128).T).astype(np.float32),
        "b2r": b2[i][None, :].astype(BF),
        "ones1": np.ones((1, 128), dtype=BF),
        "b3g": b3g,
        "biEw": biEw,
        "giEw": giEw,
    }
    return in_map, slot_entry


def kernel(values, batch_idx, gene_idx, global_latent, z, W1, b1, W2, b2, W3,
           b3):
    global LAST_RESULTS
    from concourse.bass_utils import run_bass_kernel_spmd

    batch_idx = np.asarray(batch_idx)
    gene_idx = np.asarray(gene_idx)
    global_latent = np.asarray(global_latent, dtype=np.float32)
    z = np.asarray(z, dtype=np.float32)
    W1 = np.asarray(W1, dtype=np.float32)
    b1 = np.asarray(b1, dtype=np.float32)
    W2 = np.asarray(W2, dtype=np.float32)
    b2 = np.asarray(b2, dtype=np.float32)
    W3 = np.asarray(W3, dtype=np.float32)
    b3 = np.asarray(b3, dtype=np.float32)

    nc = _get_nc()

    # Pre-transposed bf16 W3 per species (gather source tables).
    w3t_bf = [np.ascontiguousarray(W3[i].T).astype(BF) for i in range(N_SPECIES)]

    in_maps, slot_maps = [], []
    for c in range(N_CORES):
        im, se = _prep_core_inputs(c, batch_idx, gene_idx, global_latent, z,
                                   W1, b1, W2, b2, b3, w3t_bf)
        in_maps.append(im)
        slot_maps.append(se)

    LAST_RESULTS = run_bass_kernel_spmd(nc, in_maps, core_ids=list(range(N_CORES)))

    output = np.zeros(NNZ, dtype=np.float32)
    for c in range(N_CORES):
        o = np.asarray(LAST_RESULTS.results[c]["out"])  # [128, NT]
        flat = o.T.ravel()  # slot s = t*128 + p
        se = slot_maps[c]
        valid = se >= 0
        output[se[valid]] = flat[valid]
    return output



# revision 7
# speedup vs baseline: 1.1930x; 1.1930x over previous
"""Trainium2 Bass kernel for nn_Decoder (moe_routing, 4-species expert decoder).

Reference semantics (per species i, m = 4096 entries; only the first 512
decoded rows are ever read because decoded[bi, gi] indexes rows with *cell*
ids < 512):

    bi   = batch_idx[i*m:(i+1)*m]            # cell ids < 512
    gi   = gene_idx[i*m:(i+1)*m]
    comb = concat(z[i][:512], global_latent[bi[:512]])       # [512, 512]
    h1   = relu(comb @ W1[i] + b1[i])                        # [512, 1024]
    h2   = relu(h1 @ W2[i] + b2[i])                          # [512, 1024]
    out[e] = softplus(h2[bi[e]] . W3[i][:, gi[e]] + b3[i][gi[e]])

Sharding: expert-parallel x row-parallel.  Core c handles species c//2 and
MLP rows [256*(c%2), 256*(c%2+1)) plus every entry whose cell id falls in
that row range (so the two cores of a species split the MLP instead of
duplicating it).  The host pre-gathers everything indexable at input-prep
time: comb^T directly (z slice + global_latent[bi]), the W3^T rows each
entry needs (w3g), b3[gi], and a one-hot routing matrix that turns the
h2-row gather into a 128x128 matmul on the otherwise-idle PE.  Per-entry
dots are then a single fused multiply+reduce DVE pass per 128-entry chunk,
reading h2[bi] straight out of PSUM.  Math in bf16 with f32 accumulation.
"""

import os
import sys

import numpy as np

for _p in ("/root/.axon_site/_ro/trn_rl_repo", "/opt/trn_rl_repo"):
    if os.path.isdir(_p) and _p not in sys.path:
        sys.path.append(_p)

import ml_dtypes

BF = ml_dtypes.bfloat16

N_SPECIES = 4
NNZ = 16384
N_CELLS = 512
L = 256          # latent
H = 1024         # hidden
G = 20000        # genes
M = NNZ // N_SPECIES   # 4096 entries per species
R = 512          # decoded rows actually used per species
RC = 256         # rows per core (half of R)
N_CORES = 8

_NC = {}              # CHT -> compiled Bass module
LAST_RESULTS = None   # BassKernelResults of the last run (for profiling)


def _build_nc(CHT):
    """CHT = 128-entry chunks per 128-row group (2 groups per core)."""
    from contextlib import ExitStack

    import concourse.bacc as bacc
    import concourse.mybir as mybir
    import concourse.tile as tile

    F32 = mybir.dt.float32
    BF16 = mybir.dt.bfloat16
    AF = mybir.ActivationFunctionType
    OP = mybir.AluOpType

    NT = 2 * CHT

    nc = bacc.Bacc(None, target_bir_lowering=False)

    combT = nc.dram_tensor("combT", [128, 4, RC], BF16, kind="ExternalInput")
    w1 = nc.dram_tensor("w1", [128, 4, H], BF16, kind="ExternalInput")
    w2 = nc.dram_tensor("w2", [128, 8, H], BF16, kind="ExternalInput")
    b1s = nc.dram_tensor("b1s", [128, 8], F32, kind="ExternalInput")
    b2r = nc.dram_tensor("b2r", [1, H], BF16, kind="ExternalInput")
    ones1 = nc.dram_tensor("ones1", [1, 128], BF16, kind="ExternalInput")
    ohT = nc.dram_tensor("ohT", [128, 2, CHT * 128], BF16, kind="ExternalInput")
    w3g = nc.dram_tensor("w3g", [128, NT, H], BF16, kind="ExternalInput")
    b3g = nc.dram_tensor("b3g", [128, NT], F32, kind="ExternalInput")
    out = nc.dram_tensor("out", [128, NT], F32, kind="ExternalOutput")

    with tile.TileContext(nc) as tc, ExitStack() as ctx:
        const = ctx.enter_context(tc.tile_pool(name="const", bufs=1))
        work = ctx.enter_context(tc.tile_pool(name="work", bufs=1))
        scrp = ctx.enter_context(tc.tile_pool(name="scr", bufs=2))
        pmm = ctx.enter_context(tc.tile_pool(name="pmm", bufs=2, space="PSUM"))
        ph = ctx.enter_context(tc.tile_pool(name="ph", bufs=3, space="PSUM"))

        def load(src, shape, dtype, tag, eng=None):
            t = const.tile(shape, dtype, tag=tag)
            (eng or nc.sync).dma_start(t[:], src[:])
            return t

        # Small/early tensors on the sync ring, in consumption order.
        combT_s = load(combT, [128, 4, RC], BF16, "combT")
        b1_s = load(b1s, [128, 8], F32, "b1")
        b2_s = load(b2r, [1, H], BF16, "b2")
        one_s = load(ones1, [1, 128], BF16, "ones")
        ohT_s = load(ohT, [128, 2, CHT * 128], BF16, "ohT")
        b3g_s = load(b3g, [128, NT], F32, "b3g")
        # Weights on the scalar ring, split so each piece's completion lands
        # promptly (~0.5MB pieces).
        w1_s = const.tile([128, 4, H], BF16, tag="w1")
        for k2 in range(2):
            nc.scalar.dma_start(w1_s[:, 2 * k2 : 2 * (k2 + 1), :],
                                w1[:, 2 * k2 : 2 * (k2 + 1), :])
        w2_s = const.tile([128, 8, H], BF16, tag="w2")
        for k2 in range(4):
            nc.scalar.dma_start(w2_s[:, 2 * k2 : 2 * (k2 + 1), :],
                                w2[:, 2 * k2 : 2 * (k2 + 1), :])
        # Gathered W3^T rows on the sync ring (after the small tensors
        # above), in dot-consumption order.
        w3g_s = const.tile([128, NT, H], BF16, tag="w3g")
        WP = 3  # chunks per dma piece
        for tt in range(0, NT, WP):
            te = min(tt + WP, NT)
            nc.sync.dma_start(w3g_s[:, tt:te, :], w3g[:, tt:te, :])

        # h1T[h, rows]: out = W1_slice.T @ combT, relu + per-partition b1.
        h1T = work.tile([128, 8, RC], BF16, tag="h1T")
        for mt in range(8):
            ps = pmm.tile([128, RC], F32, tag="ps")
            for kt in range(4):
                nc.tensor.matmul(
                    ps[:],
                    w1_s[:, kt, mt * 128 : (mt + 1) * 128],
                    combT_s[:, kt, :],
                    start=(kt == 0),
                    stop=(kt == 3),
                )
            nc.scalar.activation(
                h1T[:, mt, :], ps[:], AF.Relu, bias=b1_s[:, mt : mt + 1]
            )

        # Per 128-row group g: h2 rows in [row, H] layout, then per 128-entry
        # chunk: hg = onehot.T @ h2rows (PE partition-permute into PSUM) and
        # a single fused DVE pass dots = b3 + sum(hg * w3row, axis=H).
        h2row = work.tile([128, 2, H], BF16, tag="h2row")
        dots = work.tile([128, NT], F32, tag="dots")
        for g in range(2):
            for nt in range(2):
                ps = pmm.tile([128, 512], F32, tag="ps")
                for kt in range(8):
                    nc.tensor.matmul(
                        ps[:],
                        h1T[:, kt, g * 128 : (g + 1) * 128],
                        w2_s[:, kt, nt * 512 : (nt + 1) * 512],
                        start=(kt == 0),
                        stop=False,
                    )
                nc.tensor.matmul(
                    ps[:],
                    one_s[:],
                    b2_s[:, nt * 512 : (nt + 1) * 512],
                    start=False,
                    stop=True,
                )
                nc.scalar.activation(
                    h2row[:, g, nt * 512 : (nt + 1) * 512], ps[:], AF.Relu
                )
            for u in range(CHT):
                t = g * CHT + u
                hp = ph.tile([128, H], F32, tag="hp")
                for nt in range(2):
                    nc.tensor.matmul(
                        hp[:, nt * 512 : (nt + 1) * 512],
                        ohT_s[:, g, u * 128 : (u + 1) * 128],
                        h2row[:, g, nt * 512 : (nt + 1) * 512],
                        start=True,
                        stop=True,
                    )
                pr = scrp.tile([128, H], BF16, tag="pr")
                nc.vector.tensor_tensor(pr[:], hp[:], w3g_s[:, t, :], OP.mult)
                nc.vector.tensor_reduce(
                    dots[:, t : t + 1], pr[:], mybir.AxisListType.X, OP.add
                )

        nc.vector.tensor_tensor(dots[:], dots[:], b3g_s[:], OP.add)

        # softplus(x) = ln(1 + e^x).  No ln/softplus in the HW act tables, so
        # compute u = e^x, y = u + 1, then ln(y) by Newton on f(l) = e^l - y:
        # l <- l + y*e^(-l) - 1, seeded with the Pade estimate 2u/(u+2).
        u = work.tile([128, NT], F32, tag="u")
        y = work.tile([128, NT], F32, tag="y")
        r = work.tile([128, NT], F32, tag="r")
        l = work.tile([128, NT], F32, tag="l")
        t_ = work.tile([128, NT], F32, tag="t_")
        nc.scalar.activation(u[:], dots[:], AF.Exp)
        nc.vector.tensor_scalar_add(y[:], u[:], 1.0)
        nc.vector.tensor_scalar_add(r[:], u[:], 2.0)
        with nc.allow_low_precision("newton seed only"):
            nc.vector.reciprocal(r[:], r[:])
        nc.vector.tensor_tensor(l[:], u[:], r[:], OP.mult)
        nc.vector.tensor_scalar_mul(l[:], l[:], 2.0)
        for _ in range(2):
            nc.scalar.activation(t_[:], l[:], AF.Exp, scale=-1.0)
            nc.vector.tensor_tensor(t_[:], y[:], t_[:], OP.mult)
            nc.vector.tensor_scalar_add(t_[:], t_[:], -1.0)
            nc.vector.tensor_tensor(l[:], l[:], t_[:], OP.add)
        nc.sync.dma_start(out[:], l[:])

    nc.finalize()
    return nc


def _get_nc(CHT):
    if CHT not in _NC:
        _NC[CHT] = _build_nc(CHT)
    return _NC[CHT]


def _prep_core_inputs(c, CHT, batch_idx, gene_idx, global_latent, z, W1, b1,
                      W2, b2, W3, b3):
    """Build the device input map for core c plus the slot->global-entry map
    used to assemble the output (slot s = t*128 + p; -1 = padding)."""
    i, j = c // 2, c % 2
    NT = 2 * CHT
    bi_sp = np.asarray(batch_idx[i * M : (i + 1) * M], dtype=np.int64)
    gi_sp = np.asarray(gene_idx[i * M : (i + 1) * M], dtype=np.int64)
    bi512 = np.asarray(batch_idx[i * M : i * M + R], dtype=np.int64)

    slot_entry = np.full(NT * 128, -1, dtype=np.int64)
    gi_slots = np.zeros(NT * 128, dtype=np.int64)
    b3_slots = np.zeros(NT * 128, dtype=np.float32)
    oh = np.zeros((128, 2, CHT * 128), dtype=BF)
    for g in range(2):
        eg = np.nonzero(bi_sp // 128 == 2 * j + g)[0]
        assert len(eg) <= CHT * 128
        ss = np.arange(len(eg))
        # slot s in group g -> tile chunk t = g*CHT + s//128, partition s%128
        slots = (g * CHT + ss // 128) * 128 + (ss % 128)
        slot_entry[slots] = i * M + eg
        gi_slots[slots] = gi_sp[eg]
        b3_slots[slots] = b3[i][gi_sp[eg]]
        oh[(bi_sp[eg] % 128), g, ss] = 1

    # W3^T rows for each slot, [128, NT, H]
    w3rows = W3[i].T[gi_slots].astype(BF)  # [NT*128, H]
    w3g = np.ascontiguousarray(
        w3rows.reshape(NT, 128, H).transpose(1, 0, 2))
    b3g = np.ascontiguousarray(b3_slots.reshape(NT, 128).T)

    # comb^T for this core's rows: feature f x row r' (r = RC*j + r')
    rows = slice(RC * j, RC * (j + 1))
    comb = np.concatenate(
        [z[i][rows], global_latent[bi512[rows]]], axis=1)  # [RC, 2L]
    combT = np.ascontiguousarray(
        comb.T.reshape(4, 128, RC).transpose(1, 0, 2)).astype(BF)

    in_map = {
        "combT": combT,
        "w1": np.ascontiguousarray(
            W1[i].reshape(4, 128, H).transpose(1, 0, 2)).astype(BF),
        "w2": np.ascontiguousarray(
            W2[i].reshape(8, 128, H).transpose(1, 0, 2)).astype(BF),
        "b1s": np.ascontiguousarray(b1[i].reshape(8, 128).T).astype(np.float32),
        "b2r": b2[i][None, :].astype(BF),
        "ones1": np.ones((1, 128), dtype=BF),
        "ohT": oh,
        "w3g": w3g,
        "b3g": b3g,
    }
    return in_map, slot_entry


def kernel(values, batch_idx, gene_idx, global_latent, z, W1, b1, W2, b2, W3,
           b3):
    global LAST_RESULTS
    from concourse.bass_utils import run_bass_kernel_spmd

    batch_idx = np.asarray(batch_idx)
    gene_idx = np.asarray(gene_idx)
    global_latent = np.asarray(global_latent, dtype=np.float32)
    z = np.asarray(z, dtype=np.float32)
    W1 = np.asarray(W1, dtype=np.float32)
    b1 = np.asarray(b1, dtype=np.float32)
    W2 = np.asarray(W2, dtype=np.float32)
    b2 = np.asarray(b2, dtype=np.float32)
    W3 = np.asarray(W3, dtype=np.float32)
    b3 = np.asarray(b3, dtype=np.float32)

    # Chunks per 128-row group: sized to the largest group so padding is <1
    # chunk; compiled kernels are cached per CHT.
    counts = np.bincount(np.asarray(batch_idx, dtype=np.int64) // 128
                         + 4 * (np.arange(NNZ) // M), minlength=16)
    CHT = int(-(-counts.max() // 128))
    nc = _get_nc(CHT)

    in_maps, slot_maps = [], []
    for c in range(N_CORES):
        im, se = _prep_core_inputs(c, CHT, batch_idx, gene_idx, global_latent,
                                   z, W1, b1, W2, b2, W3, b3)
        in_maps.append(im)
        slot_maps.append(se)

    LAST_RESULTS = run_bass_kernel_spmd(nc, in_maps, core_ids=list(range(N_CORES)))

    output = np.zeros(NNZ, dtype=np.float32)
    for c in range(N_CORES):
        o = np.asarray(LAST_RESULTS.results[c]["out"])  # [128, NT]
        flat = o.T.ravel()  # slot s = t*128 + p
        se = slot_maps[c]
        valid = se >= 0
        output[se[valid]] = flat[valid]
    return output


# revision 9
# speedup vs baseline: 1.7906x; 1.5010x over previous
"""Trainium2 Bass kernel for nn_Decoder (moe_routing, 4-species expert decoder).

Reference semantics (per species i, m = 4096 entries; only the first 512
decoded rows are ever read because decoded[bi, gi] indexes rows with *cell*
ids < 512):

    bi   = batch_idx[i*m:(i+1)*m]            # cell ids < 512
    gi   = gene_idx[i*m:(i+1)*m]
    comb = concat(z[i][:512], global_latent[bi[:512]])       # [512, 512]
    h1   = relu(comb @ W1[i] + b1[i])                        # [512, 1024]
    h2   = relu(h1 @ W2[i] + b2[i])                          # [512, 1024]
    out[e] = softplus(h2[bi[e]] . W3[i][:, gi[e]] + b3[i][gi[e]])

Sharding: expert-parallel x row-parallel.  Core c handles species c//2 and
MLP rows [256*(c%2), 256*(c%2+1)) plus every entry whose cell id falls in
that row range, so the two cores of a species split the MLP instead of
duplicating it.  The host pre-gathers everything indexable at input-prep
time: comb^T directly (z slice + global_latent[bi]), the W3 columns each
entry needs (w3gT, in [h, entry] layout), and b3[gi].  The per-entry dot
then runs on the PE: per 128-entry chunk, Q[e, c] = w3col_e . h2T[:, c]
(8 accumulating matmuls against the resident h2T), after which a one-hot
mask multiply + 128-wide reduce on the DVE picks Q[e, bi[e]].  h2 is kept
in [h, row] layout so both biases are per-partition activation biases.
Math in bf16 with f32 accumulation; no device gathers, no DRAM spills.
"""

import os
import sys

import numpy as np

for _p in ("/root/.axon_site/_ro/trn_rl_repo", "/opt/trn_rl_repo"):
    if os.path.isdir(_p) and _p not in sys.path:
        sys.path.append(_p)

import ml_dtypes

BF = ml_dtypes.bfloat16

N_SPECIES = 4
NNZ = 16384
N_CELLS = 512
L = 256          # latent
H = 1024         # hidden
G = 20000        # genes
M = NNZ // N_SPECIES   # 4096 entries per species
R = 512          # decoded rows actually used per species
RC = 256         # rows per core (half of R)
N_CORES = 8

_NC = {}              # CHT -> compiled Bass module
LAST_RESULTS = None   # BassKernelResults of the last run (for profiling)


def _build_nc(CHT):
    """CHT = 128-entry chunks per 128-row group (2 groups per core)."""
    from contextlib import ExitStack

    import concourse.bacc as bacc
    import concourse.mybir as mybir
    import concourse.tile as tile

    F32 = mybir.dt.float32
    BF16 = mybir.dt.bfloat16
    AF = mybir.ActivationFunctionType
    OP = mybir.AluOpType

    NT = 2 * CHT
    CB = 4 * RC          # combT cols in the bf16 pack
    OH0 = CB             # ohm offset in the bf16 pack

    nc = bacc.Bacc(None, target_bir_lowering=False)

    # pbf packs combT [128, 4*RC] | ohm [128, NT*128]; pf32 packs
    # b1s [128,8] | b2s [128,8] | b3g [128,NT].
    pbf = nc.dram_tensor("pbf", [128, CB + NT * 128], BF16, kind="ExternalInput")
    pf32 = nc.dram_tensor("pf32", [128, 16 + NT], F32, kind="ExternalInput")
    w1 = nc.dram_tensor("w1", [128, 4, H], BF16, kind="ExternalInput")
    w2 = nc.dram_tensor("w2", [128, 8, H], BF16, kind="ExternalInput")
    w3gT = nc.dram_tensor("w3gT", [128, NT, 8, 128], BF16, kind="ExternalInput")
    out = nc.dram_tensor("out", [128, NT], F32, kind="ExternalOutput")

    with tile.TileContext(nc) as tc, ExitStack() as ctx:
        const = ctx.enter_context(tc.tile_pool(name="const", bufs=1))
        work = ctx.enter_context(tc.tile_pool(name="work", bufs=1))
        scrp = ctx.enter_context(tc.tile_pool(name="scr", bufs=2))
        pmm = ctx.enter_context(tc.tile_pool(name="pmm", bufs=2, space="PSUM"))
        pq = ctx.enter_context(tc.tile_pool(name="pq", bufs=4, space="PSUM"))

        # Packed small tensors + w1 on the sync ring; w2 + the big gathered-W3
        # table (in dot-consumption order) on the scalar ring.
        pbf_s = const.tile([128, CB + NT * 128], BF16, tag="pbf")
        nc.sync.dma_start(pbf_s[:], pbf[:])
        pf_s = const.tile([128, 16 + NT], F32, tag="pf")
        nc.sync.dma_start(pf_s[:], pf32[:])
        w1_s = const.tile([128, 4, H], BF16, tag="w1")
        nc.sync.dma_start(w1_s[:], w1[:])
        w2_s = const.tile([128, 8, H], BF16, tag="w2")
        nc.scalar.dma_start(w2_s[:], w2[:])
        w3_s = const.tile([128, NT, 8, 128], BF16, tag="w3gT")
        WP = 3  # chunks per dma piece
        for tt in range(0, NT, WP):
            te = min(tt + WP, NT)
            nc.scalar.dma_start(w3_s[:, tt:te, :, :], w3gT[:, tt:te, :, :])

        # h1T[h, row] = relu(W1_mt.T @ combT + b1): per-partition bias.
        h1T = work.tile([128, 8, RC], BF16, tag="h1T")
        for mt in range(8):
            ps = pmm.tile([128, RC], F32, tag="ps")
            for kt in range(4):
                nc.tensor.matmul(
                    ps[:],
                    w1_s[:, kt, mt * 128 : (mt + 1) * 128],
                    pbf_s[:, kt * RC : (kt + 1) * RC],
                    start=(kt == 0),
                    stop=(kt == 3),
                )
            nc.scalar.activation(
                h1T[:, mt, :], ps[:], AF.Relu, bias=pf_s[:, mt : mt + 1]
            )

        # h2T[h, row] = relu(W2_ht.T @ h1T + b2): per-partition bias.
        h2T = work.tile([128, 8, RC], BF16, tag="h2T")
        for ht in range(8):
            ps = pmm.tile([128, RC], F32, tag="ps")
            for kt in range(8):
                nc.tensor.matmul(
                    ps[:],
                    w2_s[:, kt, ht * 128 : (ht + 1) * 128],
                    h1T[:, kt, :],
                    start=(kt == 0),
                    stop=(kt == 7),
                )
            nc.scalar.activation(
                h2T[:, ht, :], ps[:], AF.Relu, bias=pf_s[:, 8 + ht : 9 + ht]
            )

        # Per 128-entry chunk t (cell group g = t // CHT):
        #   Q[e, c] = sum_h w3gT[h, e] * h2T[h, c]   (8 matmuls, k-tiled)
        #   dots[e, t] = sum_c Q[e, c] * ohm[e, c]   (mask-pick on DVE)
        dots = work.tile([128, NT], F32, tag="dots")
        for t in range(NT):
            g = t // CHT
            q = pq.tile([128, 128], F32, tag="q")
            for kt in range(8):
                nc.tensor.matmul(
                    q[:],
                    w3_s[:, t, kt, :],
                    h2T[:, kt, g * 128 : (g + 1) * 128],
                    start=(kt == 0),
                    stop=(kt == 7),
                )
            scr = scrp.tile([128, 128], BF16, tag="scr")
            nc.vector.tensor_tensor(
                scr[:], q[:], pbf_s[:, OH0 + t * 128 : OH0 + (t + 1) * 128],
                OP.mult,
            )
            nc.vector.tensor_reduce(
                dots[:, t : t + 1], scr[:], mybir.AxisListType.X, OP.add
            )
        nc.vector.tensor_tensor(dots[:], dots[:], pf_s[:, 16 : 16 + NT], OP.add)

        # softplus(x) = ln(1 + e^x).  No ln/softplus in the HW act tables, so
        # compute u = e^x, y = u + 1, then ln(y) by Newton on f(l) = e^l - y:
        # l <- l + y*e^(-l) - 1, seeded with the Pade estimate 2u/(u+2).
        u = work.tile([128, NT], F32, tag="u")
        y = work.tile([128, NT], F32, tag="y")
        r = work.tile([128, NT], F32, tag="r")
        l = work.tile([128, NT], F32, tag="l")
        t_ = work.tile([128, NT], F32, tag="t_")
        nc.scalar.activation(u[:], dots[:], AF.Exp)
        nc.vector.tensor_scalar_add(y[:], u[:], 1.0)
        nc.vector.tensor_scalar_add(r[:], u[:], 2.0)
        with nc.allow_low_precision("newton seed only"):
            nc.vector.reciprocal(r[:], r[:])
        nc.vector.tensor_tensor(l[:], u[:], r[:], OP.mult)
        nc.vector.tensor_scalar_mul(l[:], l[:], 2.0)
        for _ in range(2):
            nc.scalar.activation(t_[:], l[:], AF.Exp, scale=-1.0)
            nc.vector.tensor_tensor(t_[:], y[:], t_[:], OP.mult)
            nc.vector.tensor_scalar_add(t_[:], t_[:], -1.0)
            nc.vector.tensor_tensor(l[:], l[:], t_[:], OP.add)
        nc.sync.dma_start(out[:], l[:])

    nc.finalize()
    return nc


def _get_nc(CHT):
    if CHT not in _NC:
        _NC[CHT] = _build_nc(CHT)
    return _NC[CHT]


def _prep_core_inputs(c, CHT, batch_idx, gene_idx, global_latent, z, W1, b1,
                      W2, b2, W3, b3):
    """Build the device input map for core c plus the slot->global-entry map
    used to assemble the output (slot s = t*128 + p; -1 = padding)."""
    i, j = c // 2, c % 2
    NT = 2 * CHT
    CB = 4 * RC
    bi_sp = np.asarray(batch_idx[i * M : (i + 1) * M], dtype=np.int64)
    gi_sp = np.asarray(gene_idx[i * M : (i + 1) * M], dtype=np.int64)
    bi512 = np.asarray(batch_idx[i * M : i * M + R], dtype=np.int64)

    slot_entry = np.full(NT * 128, -1, dtype=np.int64)
    gi_slots = np.zeros(NT * 128, dtype=np.int64)
    b3_slots = np.zeros(NT * 128, dtype=np.float32)
    ohm = np.zeros((128, NT * 128), dtype=BF)  # [entry partition, t*128 + c]
    for g in range(2):
        eg = np.nonzero(bi_sp // 128 == 2 * j + g)[0]
        assert len(eg) <= CHT * 128
        ss = np.arange(len(eg))
        tt = g * CHT + ss // 128
        pp = ss % 128
        slots = tt * 128 + pp
        slot_entry[slots] = i * M + eg
        gi_slots[slots] = gi_sp[eg]
        b3_slots[slots] = b3[i][gi_sp[eg]]
        ohm[pp, tt * 128 + (bi_sp[eg] % 128)] = 1

    # comb^T for this core's rows: feature f x row r' (r = RC*j + r')
    rows = slice(RC * j, RC * (j + 1))
    comb = np.concatenate(
        [z[i][rows], global_latent[bi512[rows]]], axis=1)  # [RC, 2L]
    combT = comb.T.astype(BF)  # [512 feat, RC]; feature f at pack col
    # pack col layout: kt*RC + r' with f = kt*128 + p
    pbf = np.empty((128, CB + NT * 128), dtype=BF)
    pbf[:, :CB] = combT.reshape(4, 128, RC).transpose(1, 0, 2).reshape(128, CB)
    pbf[:, CB:] = ohm

    pf32 = np.empty((128, 16 + NT), dtype=np.float32)
    pf32[:, 0:8] = b1[i].reshape(8, 128).T
    pf32[:, 8:16] = b2[i].reshape(8, 128).T
    pf32[:, 16:] = b3_slots.reshape(NT, 128).T

    # W3 columns for each slot in [h, entry] layout: w3gT[p, t, kt, q] =
    # W3[i][kt*128+p, gi_slots[t*128+q]]
    w3cols = W3[i][:, gi_slots].astype(BF)  # [H, NT*128]
    w3gT = np.ascontiguousarray(
        w3cols.reshape(8, 128, NT, 128).transpose(1, 2, 0, 3))

    in_map = {
        "pbf": pbf,
        "pf32": pf32,
        "w1": np.ascontiguousarray(
            W1[i].reshape(4, 128, H).transpose(1, 0, 2)).astype(BF),
        "w2": np.ascontiguousarray(
            W2[i].reshape(8, 128, H).transpose(1, 0, 2)).astype(BF),
        "w3gT": w3gT,
    }
    return in_map, slot_entry


def kernel(values, batch_idx, gene_idx, global_latent, z, W1, b1, W2, b2, W3,
           b3):
    global LAST_RESULTS
    from concourse.bass_utils import run_bass_kernel_spmd

    batch_idx = np.asarray(batch_idx)
    gene_idx = np.asarray(gene_idx)
    global_latent = np.asarray(global_latent, dtype=np.float32)
    z = np.asarray(z, dtype=np.float32)
    W1 = np.asarray(W1, dtype=np.float32)
    b1 = np.asarray(b1, dtype=np.float32)
    W2 = np.asarray(W2, dtype=np.float32)
    b2 = np.asarray(b2, dtype=np.float32)
    W3 = np.asarray(W3, dtype=np.float32)
    b3 = np.asarray(b3, dtype=np.float32)

    # Chunks per 128-row group: sized to the largest group so padding is <1
    # chunk; compiled kernels are cached per CHT.
    counts = np.bincount(np.asarray(batch_idx, dtype=np.int64) // 128
                         + 4 * (np.arange(NNZ) // M), minlength=16)
    CHT = int(-(-counts.max() // 128))
    nc = _get_nc(CHT)

    in_maps, slot_maps = [], []
    for c in range(N_CORES):
        im, se = _prep_core_inputs(c, CHT, batch_idx, gene_idx, global_latent,
                                   z, W1, b1, W2, b2, W3, b3)
        in_maps.append(im)
        slot_maps.append(se)

    LAST_RESULTS = run_bass_kernel_spmd(nc, in_maps, core_ids=list(range(N_CORES)))

    output = np.zeros(NNZ, dtype=np.float32)
    for c in range(N_CORES):
        o = np.asarray(LAST_RESULTS.results[c]["out"])  # [128, NT]
        flat = o.T.ravel()  # slot s = t*128 + p
        se = slot_maps[c]
        valid = se >= 0
        output[se[valid]] = flat[valid]
    return output


# revision 14
# speedup vs baseline: 1.9579x; 1.0934x over previous
"""Trainium2 Bass kernel for nn_Decoder (moe_routing, 4-species expert decoder).

Reference semantics (per species i, m = 4096 entries; only the first 512
decoded rows are ever read because decoded[bi, gi] indexes rows with *cell*
ids < 512):

    bi   = batch_idx[i*m:(i+1)*m]            # cell ids < 512
    gi   = gene_idx[i*m:(i+1)*m]
    comb = concat(z[i][:512], global_latent[bi[:512]])       # [512, 512]
    h1   = relu(comb @ W1[i] + b1[i])                        # [512, 1024]
    h2   = relu(h1 @ W2[i] + b2[i])                          # [512, 1024]
    out[e] = softplus(h2[bi[e]] . W3[i][:, gi[e]] + b3[i][gi[e]])

Sharding: expert-parallel x row-parallel.  Core c handles species c//2 and
MLP rows [256*(c%2), 256*(c%2+1)) plus every entry whose cell id falls in
that row range, so the two cores of a species split the MLP instead of
duplicating it.  The host pre-gathers everything indexable at input-prep
time: comb^T directly (z slice + global_latent[bi]), the W3 columns each
entry needs (w3gT, in [h, entry] layout), and b3[gi].  The per-entry dot
then runs on the PE: per 128-entry chunk, Q[e, c] = w3col_e . h2T[:, c]
(8 accumulating matmuls against the resident h2T), after which a one-hot
mask multiply + 128-wide reduce on the DVE picks Q[e, bi[e]].  h2 is kept
in [h, row] layout so both biases are per-partition activation biases.
Math in bf16 with f32 accumulation; no device gathers, no DRAM spills.
"""

import os
import sys

import numpy as np

for _p in ("/root/.axon_site/_ro/trn_rl_repo", "/opt/trn_rl_repo"):
    if os.path.isdir(_p) and _p not in sys.path:
        sys.path.append(_p)

import ml_dtypes

BF = ml_dtypes.bfloat16

N_SPECIES = 4
NNZ = 16384
N_CELLS = 512
L = 256          # latent
H = 1024         # hidden
G = 20000        # genes
M = NNZ // N_SPECIES   # 4096 entries per species
R = 512          # decoded rows actually used per species
RC = 256         # rows per core (half of R)
N_CORES = 8

_NC = {}              # CHT -> compiled Bass module
LAST_RESULTS = None   # BassKernelResults of the last run (for profiling)


def _build_nc(CHT):
    """CHT = 128-entry chunks per 128-row group (2 groups per core)."""
    from contextlib import ExitStack

    import concourse.bacc as bacc
    import concourse.mybir as mybir
    import concourse.tile as tile

    F32 = mybir.dt.float32
    BF16 = mybir.dt.bfloat16
    AF = mybir.ActivationFunctionType
    OP = mybir.AluOpType

    NT = 2 * CHT
    CB = 4 * RC          # combT cols in the bf16 pack
    OH0 = CB             # ohm offset in the bf16 pack

    nc = bacc.Bacc(None, target_bir_lowering=False)

    # pf32 packs b1s [128,8] | b2s [128,8] | b3g [128,NT].
    combT = nc.dram_tensor("combT", [128, CB], BF16, kind="ExternalInput")
    ohm = nc.dram_tensor("ohm", [128, NT * 128], BF16, kind="ExternalInput")
    pf32 = nc.dram_tensor("pf32", [128, 16 + NT], F32, kind="ExternalInput")
    w1 = nc.dram_tensor("w1", [128, 4, H], BF16, kind="ExternalInput")
    w2 = nc.dram_tensor("w2", [128, 8, H], BF16, kind="ExternalInput")
    w3gT = nc.dram_tensor("w3gT", [128, NT, 8, 128], BF16, kind="ExternalInput")
    out = nc.dram_tensor("out", [128, NT], F32, kind="ExternalOutput")

    with tile.TileContext(nc) as tc, ExitStack() as ctx:
        const = ctx.enter_context(tc.tile_pool(name="const", bufs=1))
        work = ctx.enter_context(tc.tile_pool(name="work", bufs=1))
        scrp = ctx.enter_context(tc.tile_pool(name="scr", bufs=2))
        pmm = ctx.enter_context(tc.tile_pool(name="pmm", bufs=2, space="PSUM"))
        pq = ctx.enter_context(tc.tile_pool(name="pq", bufs=4, space="PSUM"))

        # DMA priority order: the h1 prefix (combT + w1) leads the sync ring;
        # w2 + the gathered-W3 table (in dot-consumption order) stream on the
        # scalar ring; routing masks and biases follow the prefix.
        combT_s = const.tile([128, CB], BF16, tag="combT")
        nc.sync.dma_start(combT_s[:], combT[:])
        w1_s = const.tile([128, 4, H], BF16, tag="w1")
        nc.sync.dma_start(w1_s[:], w1[:])
        pf_s = const.tile([128, 16 + NT], F32, tag="pf")
        nc.sync.dma_start(pf_s[:], pf32[:])
        ohm_s = const.tile([128, NT * 128], BF16, tag="ohm")
        nc.sync.dma_start(ohm_s[:], ohm[:])
        w2_s = const.tile([128, 8, H], BF16, tag="w2")
        nc.scalar.dma_start(w2_s[:], w2[:])
        w3_s = const.tile([128, NT, 8, 128], BF16, tag="w3gT")
        WP = 3  # chunks per dma piece
        for tt in range(0, NT, WP):
            te = min(tt + WP, NT)
            nc.scalar.dma_start(w3_s[:, tt:te, :, :], w3gT[:, tt:te, :, :])

        # PE warmup while the weight DMA is in flight: ~30 throwaway matmuls
        # keep TensorE busy so it is in high-activity mode (full rate) by the
        # time h1 starts; a cold PE runs at half rate for the first ~4us.
        wu = const.tile([128, RC], BF16, tag="wu")
        nc.gpsimd.memset(wu[:], 0)
        for w in range(30):
            pw = pmm.tile([128, RC], F32, tag="ps")
            nc.tensor.matmul(pw[:], wu[:, 0:128], wu[:], start=True, stop=True)

        # h1T[h, row] = relu(W1_mt.T @ combT + b1): per-partition bias.
        h1T = work.tile([128, 8, RC], BF16, tag="h1T")
        for mt in range(8):
            ps = pmm.tile([128, RC], F32, tag="ps")
            for kt in range(4):
                nc.tensor.matmul(
                    ps[:],
                    w1_s[:, kt, mt * 128 : (mt + 1) * 128],
                    combT_s[:, kt * RC : (kt + 1) * RC],
                    start=(kt == 0),
                    stop=(kt == 3),
                )
            nc.scalar.activation(
                h1T[:, mt, :], ps[:], AF.Relu, bias=pf_s[:, mt : mt + 1]
            )

        # h2T[h, row] = relu(W2_ht.T @ h1T + b2): per-partition bias.
        h2T = work.tile([128, 8, RC], BF16, tag="h2T")
        for ht in range(8):
            ps = pmm.tile([128, RC], F32, tag="ps")
            for kt in range(8):
                nc.tensor.matmul(
                    ps[:],
                    w2_s[:, kt, ht * 128 : (ht + 1) * 128],
                    h1T[:, kt, :],
                    start=(kt == 0),
                    stop=(kt == 7),
                )
            nc.scalar.activation(
                h2T[:, ht, :], ps[:], AF.Relu, bias=pf_s[:, 8 + ht : 9 + ht]
            )

        # Per 128-entry chunk t (cell group g = t // CHT):
        #   Q[e, c] = sum_h w3gT[h, e] * h2T[h, c]   (8 matmuls, k-tiled)
        #   dots[e, t] = sum_c Q[e, c] * ohm[e, c]   (mask-pick on DVE)
        dots = work.tile([128, NT], F32, tag="dots")
        for t in range(NT):
            g = t // CHT
            q = pq.tile([128, 128], F32, tag="q")
            for kt in range(8):
                nc.tensor.matmul(
                    q[:],
                    w3_s[:, t, kt, :],
                    h2T[:, kt, g * 128 : (g + 1) * 128],
                    start=(kt == 0),
                    stop=(kt == 7),
                )
            scr = scrp.tile([128, 128], BF16, tag="scr")
            nc.vector.tensor_tensor(
                scr[:], q[:], ohm_s[:, t * 128 : (t + 1) * 128],
                OP.mult,
            )
            nc.vector.tensor_reduce(
                dots[:, t : t + 1], scr[:], mybir.AxisListType.X, OP.add
            )
        nc.vector.tensor_tensor(dots[:], dots[:], pf_s[:, 16 : 16 + NT], OP.add)

        # softplus(x) = ln(1 + e^x).  No ln/softplus in the HW act tables, so
        # compute u = e^x, y = u + 1, then ln(y) by Newton on f(l) = e^l - y:
        # l <- l + y*e^(-l) - 1, seeded with the Pade estimate 2u/(u+2).
        u = work.tile([128, NT], F32, tag="u")
        y = work.tile([128, NT], F32, tag="y")
        r = work.tile([128, NT], F32, tag="r")
        l = work.tile([128, NT], F32, tag="l")
        t_ = work.tile([128, NT], F32, tag="t_")
        nc.scalar.activation(u[:], dots[:], AF.Exp)
        nc.vector.tensor_scalar_add(y[:], u[:], 1.0)
        nc.vector.tensor_scalar_add(r[:], u[:], 2.0)
        with nc.allow_low_precision("newton seed only"):
            nc.vector.reciprocal(r[:], r[:])
        nc.vector.tensor_tensor(l[:], u[:], r[:], OP.mult)
        nc.vector.tensor_scalar_mul(l[:], l[:], 2.0)
        for _ in range(2):
            nc.scalar.activation(t_[:], l[:], AF.Exp, scale=-1.0)
            nc.vector.tensor_tensor(t_[:], y[:], t_[:], OP.mult)
            nc.vector.tensor_scalar_add(t_[:], t_[:], -1.0)
            nc.vector.tensor_tensor(l[:], l[:], t_[:], OP.add)
        nc.sync.dma_start(out[:], l[:])

    nc.finalize()
    return nc


def _get_nc(CHT):
    if CHT not in _NC:
        _NC[CHT] = _build_nc(CHT)
    return _NC[CHT]


def _prep_core_inputs(c, CHT, batch_idx, gene_idx, global_latent, z, W1, b1,
                      W2, b2, W3, b3):
    """Build the device input map for core c plus the slot->global-entry map
    used to assemble the output (slot s = t*128 + p; -1 = padding)."""
    i, j = c // 2, c % 2
    NT = 2 * CHT
    CB = 4 * RC
    bi_sp = np.asarray(batch_idx[i * M : (i + 1) * M], dtype=np.int64)
    gi_sp = np.asarray(gene_idx[i * M : (i + 1) * M], dtype=np.int64)
    bi512 = np.asarray(batch_idx[i * M : i * M + R], dtype=np.int64)

    slot_entry = np.full(NT * 128, -1, dtype=np.int64)
    gi_slots = np.zeros(NT * 128, dtype=np.int64)
    b3_slots = np.zeros(NT * 128, dtype=np.float32)
    ohm = np.zeros((128, NT * 128), dtype=BF)  # [entry partition, t*128 + c]
    for g in range(2):
        eg = np.nonzero(bi_sp // 128 == 2 * j + g)[0]
        assert len(eg) <= CHT * 128
        ss = np.arange(len(eg))
        tt = g * CHT + ss // 128
        pp = ss % 128
        slots = tt * 128 + pp
        slot_entry[slots] = i * M + eg
        gi_slots[slots] = gi_sp[eg]
        b3_slots[slots] = b3[i][gi_sp[eg]]
        ohm[pp, tt * 128 + (bi_sp[eg] % 128)] = 1

    # comb^T for this core's rows: feature f x row r' (r = RC*j + r')
    rows = slice(RC * j, RC * (j + 1))
    comb = np.concatenate(
        [z[i][rows], global_latent[bi512[rows]]], axis=1)  # [RC, 2L]
    combT = comb.T.astype(BF)  # [512 feat, RC]
    # col layout: kt*RC + r' with feature f = kt*128 + p
    combT_pk = np.ascontiguousarray(
        combT.reshape(4, 128, RC).transpose(1, 0, 2).reshape(128, CB))

    pf32 = np.empty((128, 16 + NT), dtype=np.float32)
    pf32[:, 0:8] = b1[i].reshape(8, 128).T
    pf32[:, 8:16] = b2[i].reshape(8, 128).T
    pf32[:, 16:] = b3_slots.reshape(NT, 128).T

    # W3 columns for each slot in [h, entry] layout: w3gT[p, t, kt, q] =
    # W3[i][kt*128+p, gi_slots[t*128+q]]
    w3cols = W3[i][:, gi_slots].astype(BF)  # [H, NT*128]
    w3gT = np.ascontiguousarray(
        w3cols.reshape(8, 128, NT, 128).transpose(1, 2, 0, 3))

    in_map = {
        "combT": combT_pk,
        "ohm": ohm,
        "pf32": pf32,
        "w1": np.ascontiguousarray(
            W1[i].reshape(4, 128, H).transpose(1, 0, 2)).astype(BF),
        "w2": np.ascontiguousarray(
            W2[i].reshape(8, 128, H).transpose(1, 0, 2)).astype(BF),
        "w3gT": w3gT,
    }
    return in_map, slot_entry


def kernel(values, batch_idx, gene_idx, global_latent, z, W1, b1, W2, b2, W3,
           b3):
    global LAST_RESULTS
    from concourse.bass_utils import run_bass_kernel_spmd

    batch_idx = np.asarray(batch_idx)
    gene_idx = np.asarray(gene_idx)
    global_latent = np.asarray(global_latent, dtype=np.float32)
    z = np.asarray(z, dtype=np.float32)
    W1 = np.asarray(W1, dtype=np.float32)
    b1 = np.asarray(b1, dtype=np.float32)
    W2 = np.asarray(W2, dtype=np.float32)
    b2 = np.asarray(b2, dtype=np.float32)
    W3 = np.asarray(W3, dtype=np.float32)
    b3 = np.asarray(b3, dtype=np.float32)

    # Chunks per 128-row group: sized to the largest group so padding is <1
    # chunk; compiled kernels are cached per CHT.
    counts = np.bincount(np.asarray(batch_idx, dtype=np.int64) // 128
                         + 4 * (np.arange(NNZ) // M), minlength=16)
    CHT = int(-(-counts.max() // 128))
    nc = _get_nc(CHT)

    in_maps, slot_maps = [], []
    for c in range(N_CORES):
        im, se = _prep_core_inputs(c, CHT, batch_idx, gene_idx, global_latent,
                                   z, W1, b1, W2, b2, W3, b3)
        in_maps.append(im)
        slot_maps.append(se)

    LAST_RESULTS = run_bass_kernel_spmd(nc, in_maps, core_ids=list(range(N_CORES)))

    output = np.zeros(NNZ, dtype=np.float32)
    for c in range(N_CORES):
        o = np.asarray(LAST_RESULTS.results[c]["out"])  # [128, NT]
        flat = o.T.ravel()  # slot s = t*128 + p
        se = slot_maps[c]
        valid = se >= 0
        output[se[valid]] = flat[valid]
    return output


# revision 16
# speedup vs baseline: 2.1102x; 1.0778x over previous
"""Trainium2 Bass kernel for nn_Decoder (moe_routing, 4-species expert decoder).

Reference semantics (per species i, m = 4096 entries; only the first 512
decoded rows are ever read because decoded[bi, gi] indexes rows with *cell*
ids < 512):

    bi   = batch_idx[i*m:(i+1)*m]            # cell ids < 512
    gi   = gene_idx[i*m:(i+1)*m]
    comb = concat(z[i][:512], global_latent[bi[:512]])       # [512, 512]
    h1   = relu(comb @ W1[i] + b1[i])                        # [512, 1024]
    h2   = relu(h1 @ W2[i] + b2[i])                          # [512, 1024]
    out[e] = softplus(h2[bi[e]] . W3[i][:, gi[e]] + b3[i][gi[e]])

Sharding: expert-parallel x row-parallel.  Core c handles species c//2 and
MLP rows [256*(c%2), 256*(c%2+1)) plus every entry whose cell id falls in
that row range, so the two cores of a species split the MLP instead of
duplicating it.  The host pre-gathers everything indexable at input-prep
time: comb^T directly (z slice + global_latent[bi]), the W3 columns each
entry needs (w3gT, in [h, entry] layout), and b3[gi].  The per-entry dot
then runs on the PE: per 128-entry chunk, Q[e, c] = w3col_e . h2T[:, c]
(8 accumulating matmuls against the resident h2T), after which a one-hot
mask multiply + 128-wide reduce on the DVE picks Q[e, bi[e]].  h2 is kept
in [h, row] layout so both biases are per-partition activation biases.
Math in bf16 with f32 accumulation; no device gathers, no DRAM spills.
"""

import os
import sys

import numpy as np

for _p in ("/root/.axon_site/_ro/trn_rl_repo", "/opt/trn_rl_repo"):
    if os.path.isdir(_p) and _p not in sys.path:
        sys.path.append(_p)

import ml_dtypes

BF = ml_dtypes.bfloat16

N_SPECIES = 4
NNZ = 16384
N_CELLS = 512
L = 256          # latent
H = 1024         # hidden
G = 20000        # genes
M = NNZ // N_SPECIES   # 4096 entries per species
R = 512          # decoded rows actually used per species
RC = 256         # rows per core (half of R)
N_CORES = 8

_NC = {}              # CHT -> compiled Bass module
LAST_RESULTS = None   # BassKernelResults of the last run (for profiling)


def _build_nc(CHT):
    """CHT = 128-entry chunks per 128-row group (2 groups per core)."""
    from contextlib import ExitStack

    import concourse.bacc as bacc
    import concourse.mybir as mybir
    import concourse.tile as tile

    F32 = mybir.dt.float32
    BF16 = mybir.dt.bfloat16
    AF = mybir.ActivationFunctionType
    OP = mybir.AluOpType

    NT = 2 * CHT
    CB = 4 * RC          # combT cols in the bf16 pack
    OH0 = CB             # ohm offset in the bf16 pack

    nc = bacc.Bacc(None, target_bir_lowering=False)

    # pf32 packs b1s [128,8] | b2s [128,8] | b3g [128,NT].
    combT = nc.dram_tensor("combT", [128, CB], BF16, kind="ExternalInput")
    ohm = nc.dram_tensor("ohm", [128, NT * 128], BF16, kind="ExternalInput")
    pf32 = nc.dram_tensor("pf32", [128, 16 + NT], F32, kind="ExternalInput")
    w1r = nc.dram_tensor("w1r", [128, 8, 4, 128], BF16, kind="ExternalInput")
    w2r = nc.dram_tensor("w2r", [128, 8, 8, 128], BF16, kind="ExternalInput")
    w3gT = nc.dram_tensor("w3gT", [128, NT, 8, 128], BF16, kind="ExternalInput")
    out = nc.dram_tensor("out", [128, NT], F32, kind="ExternalOutput")

    with tile.TileContext(nc) as tc, ExitStack() as ctx:
        const = ctx.enter_context(tc.tile_pool(name="const", bufs=1))
        work = ctx.enter_context(tc.tile_pool(name="work", bufs=1))
        scrp = ctx.enter_context(tc.tile_pool(name="scr", bufs=2))
        pmm = ctx.enter_context(tc.tile_pool(name="pmm", bufs=2, space="PSUM"))
        pwu = ctx.enter_context(tc.tile_pool(name="pwu", bufs=2, space="PSUM"))
        pq = ctx.enter_context(tc.tile_pool(name="pq", bufs=4, space="PSUM"))

        # All loads ride the sync ring in strict consumption-priority order,
        # each piece its own tile so compute can start the moment its piece
        # lands: combT, w1 halves, biases, w2 halves, masks, then the W3
        # table streamed in dot order.  (The scalar ring stays free so relus
        # are never stuck behind DIRECT2D issue.)
        combT_s = const.tile([128, CB], BF16, tag="combT")
        nc.sync.dma_start(combT_s[:], combT[:])
        w1_t = []
        for h in range(2):
            w1_t.append(const.tile([128, 4, 4, 128], BF16, name=f"w1{h}", tag=f"w1{h}"))
            nc.sync.dma_start(w1_t[h][:], w1r[:, 4 * h : 4 * (h + 1), :, :])
        pf_s = const.tile([128, 16 + NT], F32, tag="pf")
        nc.sync.dma_start(pf_s[:], pf32[:])
        w2_t = []
        for h in range(2):
            w2_t.append(const.tile([128, 4, 8, 128], BF16, name=f"w2{h}", tag=f"w2{h}"))
            nc.sync.dma_start(w2_t[h][:], w2r[:, 4 * h : 4 * (h + 1), :, :])
        ohm_s = const.tile([128, NT * 128], BF16, tag="ohm")
        nc.sync.dma_start(ohm_s[:], ohm[:])
        WP = 3  # chunks per dma piece
        w3_t = []
        for pi in range(0, NT, WP):
            pe_ = min(pi + WP, NT)
            tw = const.tile([128, pe_ - pi, 8, 128], BF16, name=f"w3p{pi}", tag=f"w3p{pi}")
            nc.sync.dma_start(tw[:], w3gT[:, pi:pe_, :, :])
            w3_t.append(tw)

        # PE warmup while the weight DMA is in flight: throwaway matmuls keep
        # TensorE in high-activity mode (a cold PE runs at half rate).
        wu = const.tile([128, RC], BF16, tag="wu")
        nc.gpsimd.memset(wu[:], 0)
        for w in range(10):
            pw = pwu.tile([128, RC], F32, tag="pw")
            nc.tensor.matmul(pw[:], wu[:, 0:128], wu[:], start=True, stop=True)

        # h1T[h, row] = relu(W1_mt.T @ combT + b1): per-partition bias.
        h1T = work.tile([128, 8, RC], BF16, tag="h1T")
        for mt in range(8):
            ps = pmm.tile([128, RC], F32, tag="ps")
            for kt in range(4):
                nc.tensor.matmul(
                    ps[:],
                    w1_t[mt // 4][:, mt % 4, kt, :],
                    combT_s[:, kt * RC : (kt + 1) * RC],
                    start=(kt == 0),
                    stop=(kt == 3),
                )
            nc.scalar.activation(
                h1T[:, mt, :], ps[:], AF.Relu, bias=pf_s[:, mt : mt + 1]
            )

        # h2T[h, row] = relu(W2_ht.T @ h1T + b2): per-partition bias.
        h2T = work.tile([128, 8, RC], BF16, tag="h2T")
        for ht in range(8):
            ps = pmm.tile([128, RC], F32, tag="ps")
            for kt in range(8):
                nc.tensor.matmul(
                    ps[:],
                    w2_t[ht // 4][:, ht % 4, kt, :],
                    h1T[:, kt, :],
                    start=(kt == 0),
                    stop=(kt == 7),
                )
            nc.scalar.activation(
                h2T[:, ht, :], ps[:], AF.Relu, bias=pf_s[:, 8 + ht : 9 + ht]
            )

        # Per 128-entry chunk t (cell group g = t // CHT):
        #   Q[e, c] = sum_h w3gT[h, e] * h2T[h, c]   (8 matmuls, k-tiled)
        #   dots[e, t] = sum_c Q[e, c] * ohm[e, c]   (mask-pick on DVE)
        dots = work.tile([128, NT], F32, tag="dots")
        for t in range(NT):
            g = t // CHT
            q = pq.tile([128, 128], F32, tag="q")
            for kt in range(8):
                nc.tensor.matmul(
                    q[:],
                    w3_t[t // WP][:, t % WP, kt, :],
                    h2T[:, kt, g * 128 : (g + 1) * 128],
                    start=(kt == 0),
                    stop=(kt == 7),
                )
            scr = scrp.tile([128, 128], BF16, tag="scr")
            nc.vector.tensor_tensor(
                scr[:], q[:], ohm_s[:, t * 128 : (t + 1) * 128],
                OP.mult,
            )
            nc.vector.tensor_reduce(
                dots[:, t : t + 1], scr[:], mybir.AxisListType.X, OP.add
            )
        nc.vector.tensor_tensor(dots[:], dots[:], pf_s[:, 16 : 16 + NT], OP.add)

        # softplus(x) = ln(1 + e^x).  No ln/softplus in the HW act tables, so
        # compute u = e^x, y = u + 1, then ln(y) by Newton on f(l) = e^l - y:
        # l <- l + y*e^(-l) - 1, seeded with the Pade estimate 2u/(u+2).
        u = work.tile([128, NT], F32, tag="u")
        y = work.tile([128, NT], F32, tag="y")
        r = work.tile([128, NT], F32, tag="r")
        l = work.tile([128, NT], F32, tag="l")
        t_ = work.tile([128, NT], F32, tag="t_")
        nc.scalar.activation(u[:], dots[:], AF.Exp)
        nc.vector.tensor_scalar_add(y[:], u[:], 1.0)
        nc.vector.tensor_scalar_add(r[:], u[:], 2.0)
        with nc.allow_low_precision("newton seed only"):
            nc.vector.reciprocal(r[:], r[:])
        nc.vector.tensor_tensor(l[:], u[:], r[:], OP.mult)
        nc.vector.tensor_scalar_mul(l[:], l[:], 2.0)
        for _ in range(2):
            nc.scalar.activation(t_[:], l[:], AF.Exp, scale=-1.0)
            nc.vector.tensor_tensor(t_[:], y[:], t_[:], OP.mult)
            nc.vector.tensor_scalar_add(t_[:], t_[:], -1.0)
            nc.vector.tensor_tensor(l[:], l[:], t_[:], OP.add)
        nc.sync.dma_start(out[:], l[:])

    nc.finalize()
    return nc


def _get_nc(CHT):
    if CHT not in _NC:
        _NC[CHT] = _build_nc(CHT)
    return _NC[CHT]


def _prep_core_inputs(c, CHT, batch_idx, gene_idx, global_latent, z, W1, b1,
                      W2, b2, W3, b3):
    """Build the device input map for core c plus the slot->global-entry map
    used to assemble the output (slot s = t*128 + p; -1 = padding)."""
    i, j = c // 2, c % 2
    NT = 2 * CHT
    CB = 4 * RC
    bi_sp = np.asarray(batch_idx[i * M : (i + 1) * M], dtype=np.int64)
    gi_sp = np.asarray(gene_idx[i * M : (i + 1) * M], dtype=np.int64)
    bi512 = np.asarray(batch_idx[i * M : i * M + R], dtype=np.int64)

    slot_entry = np.full(NT * 128, -1, dtype=np.int64)
    gi_slots = np.zeros(NT * 128, dtype=np.int64)
    b3_slots = np.zeros(NT * 128, dtype=np.float32)
    ohm = np.zeros((128, NT * 128), dtype=BF)  # [entry partition, t*128 + c]
    for g in range(2):
        eg = np.nonzero(bi_sp // 128 == 2 * j + g)[0]
        assert len(eg) <= CHT * 128
        ss = np.arange(len(eg))
        tt = g * CHT + ss // 128
        pp = ss % 128
        slots = tt * 128 + pp
        slot_entry[slots] = i * M + eg
        gi_slots[slots] = gi_sp[eg]
        b3_slots[slots] = b3[i][gi_sp[eg]]
        ohm[pp, tt * 128 + (bi_sp[eg] % 128)] = 1

    # comb^T for this core's rows: feature f x row r' (r = RC*j + r')
    rows = slice(RC * j, RC * (j + 1))
    comb = np.concatenate(
        [z[i][rows], global_latent[bi512[rows]]], axis=1)  # [RC, 2L]
    combT = comb.T.astype(BF)  # [512 feat, RC]
    # col layout: kt*RC + r' with feature f = kt*128 + p
    combT_pk = np.ascontiguousarray(
        combT.reshape(4, 128, RC).transpose(1, 0, 2).reshape(128, CB))

    pf32 = np.empty((128, 16 + NT), dtype=np.float32)
    pf32[:, 0:8] = b1[i].reshape(8, 128).T
    pf32[:, 8:16] = b2[i].reshape(8, 128).T
    pf32[:, 16:] = b3_slots.reshape(NT, 128).T

    # W3 columns for each slot in [h, entry] layout: w3gT[p, t, kt, q] =
    # W3[i][kt*128+p, gi_slots[t*128+q]]
    w3cols = W3[i][:, gi_slots].astype(BF)  # [H, NT*128]
    w3gT = np.ascontiguousarray(
        w3cols.reshape(8, 128, NT, 128).transpose(1, 2, 0, 3))

    in_map = {
        "combT": combT_pk,
        "ohm": ohm,
        "pf32": pf32,
        "w1r": np.ascontiguousarray(
            W1[i].reshape(4, 128, 8, 128).transpose(1, 2, 0, 3)).astype(BF),
        "w2r": np.ascontiguousarray(
            W2[i].reshape(8, 128, 8, 128).transpose(1, 2, 0, 3)).astype(BF),
        "w3gT": w3gT,
    }
    return in_map, slot_entry


def kernel(values, batch_idx, gene_idx, global_latent, z, W1, b1, W2, b2, W3,
           b3):
    global LAST_RESULTS
    from concourse.bass_utils import run_bass_kernel_spmd

    batch_idx = np.asarray(batch_idx)
    gene_idx = np.asarray(gene_idx)
    global_latent = np.asarray(global_latent, dtype=np.float32)
    z = np.asarray(z, dtype=np.float32)
    W1 = np.asarray(W1, dtype=np.float32)
    b1 = np.asarray(b1, dtype=np.float32)
    W2 = np.asarray(W2, dtype=np.float32)
    b2 = np.asarray(b2, dtype=np.float32)
    W3 = np.asarray(W3, dtype=np.float32)
    b3 = np.asarray(b3, dtype=np.float32)

    # Chunks per 128-row group: sized to the largest group so padding is <1
    # chunk; compiled kernels are cached per CHT.
    counts = np.bincount(np.asarray(batch_idx, dtype=np.int64) // 128
                         + 4 * (np.arange(NNZ) // M), minlength=16)
    CHT = int(-(-counts.max() // 128))
    nc = _get_nc(CHT)

    in_maps, slot_maps = [], []
    for c in range(N_CORES):
        im, se = _prep_core_inputs(c, CHT, batch_idx, gene_idx, global_latent,
                                   z, W1, b1, W2, b2, W3, b3)
        in_maps.append(im)
        slot_maps.append(se)

    LAST_RESULTS = run_bass_kernel_spmd(nc, in_maps, core_ids=list(range(N_CORES)))

    output = np.zeros(NNZ, dtype=np.float32)
    for c in range(N_CORES):
        o = np.asarray(LAST_RESULTS.results[c]["out"])  # [128, NT]
        flat = o.T.ravel()  # slot s = t*128 + p
        se = slot_maps[c]
        valid = se >= 0
        output[se[valid]] = flat[valid]
    return output
